# revision 2
# baseline (speedup 1.0000x reference)
"""ConvLSTM attention pooling kernel for 8 Trainium2 NeuronCores.

Reference computation (per sample b):
    frames = x[b].reshape(chi, D)            # D = C*H*W = 65536, chi = 20
    scores = frames @ frames[-1] / chi       # [chi]
    alpha  = softmax(scores)                 # [chi]
    y      = x[b].reshape(D, chi) @ alpha    # [D]  (interleaved view!)

Sharding: pure data-parallel over batch B=64 -> 8 samples per core.

Per-core kernel (per sample):
  - load x[b] in "frames" layout  t1[p, c, f] = u[c*65536 + p*512 + f]
  - 20x fused multiply+reduce (DVE) -> per-partition partial dots [128, 20]
  - cross-partition sum via PE matmul with a (1/chi)-constant vector -> scores/chi
  - softmax on [1, 20] (DVE max / ACT exp+accum / DVE reciprocal)
  - broadcast alpha to 128 partitions via PE matmul with ones
  - load x[b] in "interleaved" layout t2[p, f2, c] = u[(p*512+f2)*20 + c]
  - 20x fused FMA chain (DVE scalar_tensor_tensor): out += t2[:, :, c] * alpha[c]
  - store out[p, f2] -> y[b, p*512 + f2]
"""

import numpy as np

B = 64
CHI = 20
D = 64 * 32 * 32  # 65536
N_CORES = 8
S = B // N_CORES  # samples per core
P = 128
F = D // P  # 512

_CACHE = {}


def _build_nc():
    import concourse.bacc as bacc
    import concourse.tile as tile
    from concourse import mybir

    f32 = mybir.dt.float32
    nc = bacc.Bacc("TRN2", target_bir_lowering=False, debug=False)
    x_d = nc.dram_tensor("x", [S, CHI * D], f32, kind="ExternalInput").ap()
    y_d = nc.dram_tensor("y", [S, D], f32, kind="ExternalOutput").ap()

    with tile.TileContext(nc) as tc:
        with (
            tc.tile_pool(name="t1", bufs=2) as t1_pool,
            tc.tile_pool(name="t2", bufs=2) as t2_pool,
            tc.tile_pool(name="scratch", bufs=2) as s_pool,
            tc.tile_pool(name="small", bufs=4) as sm_pool,
            tc.tile_pool(name="outp", bufs=2) as o_pool,
            tc.tile_pool(name="singles", bufs=1) as ones_pool,
            tc.tile_pool(name="psum", bufs=2, space="PSUM") as p_pool,
        ):
            inv_chi_col = ones_pool.tile([P, 1], f32)
            nc.vector.memset(inv_chi_col, 1.0 / CHI)
            ones_row = ones_pool.tile([1, P], f32)
            nc.vector.memset(ones_row, 1.0)

            for b in range(S):
                u = x_d[b]
                # frames layout: [p, c, f] <- u[c*D + p*F + f]
                t1 = t1_pool.tile([P, CHI, F], f32)
                nc.sync.dma_start(
                    out=t1, in_=u.rearrange("(c p f) -> p c f", p=P, f=F)
                )
                # interleaved layout: [p, f2, c] <- u[(p*F + f2)*CHI + c]
                t2 = t2_pool.tile([P, F, CHI], f32)
                nc.sync.dma_start(
                    out=t2, in_=u.rearrange("(p f c) -> p f c", p=P, c=CHI)
                )

                # ---- stage 1: scores ----
                partials = sm_pool.tile([P, CHI], f32)
                scratch = s_pool.tile([P, F], f32)
                for c in range(CHI):
                    # fused multiply + free-dim reduce: out = (in0 * 1) * in1,
                    # accum_out = sum(out) per partition
                    nc.vector.scalar_tensor_tensor(
                        out=scratch,
                        in0=t1[:, c, :],
                        scalar=1.0,
                        in1=t1[:, CHI - 1, :],
                        op0=mybir.AluOpType.mult,
                        op1=mybir.AluOpType.mult,
                        accum_out=partials[:, c : c + 1],
                    )

                s_psum = p_pool.tile([1, CHI], f32)
                nc.tensor.matmul(s_psum, inv_chi_col, partials, start=True, stop=True)
                scores = sm_pool.tile([1, CHI], f32)
                nc.vector.tensor_copy(out=scores, in_=s_psum)

                # ---- softmax on [1, CHI] ----
                neg_mx = sm_pool.tile([1, 1], f32)
                nc.vector.tensor_reduce(
                    out=neg_mx,
                    in_=scores,
                    axis=mybir.AxisListType.X,
                    op=mybir.AluOpType.max,
                    negate=True,
                )
                exps = sm_pool.tile([1, CHI], f32)
                sumexp = sm_pool.tile([1, 1], f32)
                nc.scalar.activation(
                    out=exps,
                    in_=scores,
                    func=mybir.ActivationFunctionType.Exp,
                    bias=neg_mx[:, 0:1],
                    scale=1.0,
                    accum_out=sumexp,
                )
                rsum = sm_pool.tile([1, 1], f32)
                nc.vector.reciprocal(rsum, sumexp)
                alpha = sm_pool.tile([1, CHI], f32)
                nc.vector.tensor_scalar_mul(alpha, exps, rsum)

                # broadcast alpha to all partitions: [128, CHI]
                b_psum = p_pool.tile([P, CHI], f32)
                nc.tensor.matmul(b_psum, ones_row, alpha, start=True, stop=True)
                alpha_bc = sm_pool.tile([P, CHI], f32)
                nc.scalar.copy(out=alpha_bc, in_=b_psum)

                # ---- stage 2: weighted sum over interleaved view ----
                out_t = o_pool.tile([P, F], f32)
                nc.vector.tensor_scalar_mul(out_t, t2[:, :, 0], alpha_bc[:, 0:1])
                for c in range(1, CHI):
                    nc.vector.scalar_tensor_tensor(
                        out=out_t,
                        in0=t2[:, :, c],
                        scalar=alpha_bc[:, c : c + 1],
                        in1=out_t,
                        op0=mybir.AluOpType.mult,
                        op1=mybir.AluOpType.add,
                    )

                nc.sync.dma_start(
                    out=y_d[b].rearrange("(p f) -> p f", p=P), in_=out_t
                )

    nc.compile()
    return nc


def _get_nc():
    if "nc" not in _CACHE:
        _CACHE["nc"] = _build_nc()
    return _CACHE["nc"]


def kernel(**inputs):
    from concourse.bass_utils import run_bass_kernel_spmd

    x = np.ascontiguousarray(np.asarray(inputs["x"], dtype=np.float32))
    assert x.shape == (B, CHI, 64, 32, 32), x.shape
    xs = x.reshape(B, CHI * D)
    nc = _get_nc()
    in_maps = [
        {"x": xs[i * S : (i + 1) * S]} for i in range(N_CORES)
    ]
    last_err = None
    for _attempt in range(3):
        try:
            res = run_bass_kernel_spmd(nc, in_maps, core_ids=list(range(N_CORES)))
            break
        except Exception as e:  # transient NRT device errors: retry
            last_err = e
    else:
        raise last_err
    out = np.concatenate([res.results[i]["y"] for i in range(N_CORES)], axis=0)
    return out.reshape(B, 64, 32, 32)


# revision 8
# speedup vs baseline: 59.0567x; 59.0567x over previous
"""ConvLSTM attention pooling kernel for 8 Trainium2 NeuronCores.

Reference computation (per sample b):
    frames = x[b].reshape(chi, D)            # D = C*H*W = 65536, chi = 20
    scores = frames @ frames[-1] / chi       # [chi]
    alpha  = softmax(scores)                 # [chi]
    y      = x[b].reshape(D, chi) @ alpha    # [D]  (interleaved view!)

Sharding: pure data-parallel over batch B=64 -> 8 samples per core.

Per-core kernel (per sample):
  - load x[b] in "frames" layout  t1[p, c, f] = u[c*65536 + p*512 + f]
  - 20x fused multiply+reduce (DVE) -> per-partition partial dots [128, 20]
  - cross-partition sum via PE matmul with a (1/chi)-constant vector -> scores/chi
  - softmax on [1, 20] (DVE max / ACT exp+accum / DVE reciprocal)
  - broadcast alpha to 128 partitions via PE matmul with ones
  - load x[b] in "interleaved" layout t2[p, f2, c] = u[(p*512+f2)*20 + c]
  - 20x fused FMA chain (DVE scalar_tensor_tensor): out += t2[:, :, c] * alpha[c]
  - store out[p, f2] -> y[b, p*512 + f2]
"""

import numpy as np

B = 64
CHI = 20
D = 64 * 32 * 32  # 65536
N_CORES = 8
S = B // N_CORES  # samples per core
P = 128
F = D // P  # 512

_CACHE = {}


def _build_nc():
    import concourse.bacc as bacc
    import concourse.tile as tile
    from concourse import mybir

    f32 = mybir.dt.float32
    nc = bacc.Bacc("TRN2", target_bir_lowering=False, debug=False)
    x_d = nc.dram_tensor("x", [S, CHI * D], f32, kind="ExternalInput").ap()
    y_d = nc.dram_tensor("y", [S, D], f32, kind="ExternalOutput").ap()

    with tile.TileContext(nc) as tc:
        with (
            tc.tile_pool(name="t1", bufs=2) as t1_pool,
            tc.tile_pool(name="t2", bufs=2) as t2_pool,
            tc.tile_pool(name="scratch", bufs=2) as s_pool,
            tc.tile_pool(name="small", bufs=4) as sm_pool,
            tc.tile_pool(name="outp", bufs=2) as o_pool,
            tc.tile_pool(name="singles", bufs=1) as ones_pool,
            tc.tile_pool(name="psum", bufs=2, space="PSUM") as p_pool,
        ):
            inv_chi_col = ones_pool.tile([P, 1], f32)
            nc.vector.memset(inv_chi_col, 1.0 / CHI)
            ones_row = ones_pool.tile([1, P], f32)
            nc.vector.memset(ones_row, 1.0)

            for b in range(S):
                u = x_d[b]
                # frames layout: [p, c, f] <- u[c*D + p*F + f]
                t1 = t1_pool.tile([P, CHI, F], f32)
                nc.sync.dma_start(
                    out=t1, in_=u.rearrange("(c p f) -> p c f", p=P, f=F)
                )
                # interleaved layout: [p, f2, c] <- u[(p*F + f2)*CHI + c]
                t2 = t2_pool.tile([P, F, CHI], f32)
                nc.sync.dma_start(
                    out=t2, in_=u.rearrange("(p f c) -> p f c", p=P, c=CHI)
                )

                # ---- stage 1: scores ----
                partials = sm_pool.tile([P, CHI], f32)
                scratch = s_pool.tile([P, F], f32)
                for c in range(CHI):
                    # fused multiply + free-dim reduce: out = (in0 * 1) * in1,
                    # accum_out = sum(out) per partition
                    nc.vector.scalar_tensor_tensor(
                        out=scratch,
                        in0=t1[:, c, :],
                        scalar=1.0,
                        in1=t1[:, CHI - 1, :],
                        op0=mybir.AluOpType.mult,
                        op1=mybir.AluOpType.mult,
                        accum_out=partials[:, c : c + 1],
                    )

                s_psum = p_pool.tile([1, CHI], f32)
                nc.tensor.matmul(s_psum, inv_chi_col, partials, start=True, stop=True)
                scores = sm_pool.tile([1, CHI], f32)
                nc.vector.tensor_copy(out=scores, in_=s_psum)

                # ---- softmax on [1, CHI] ----
                neg_mx = sm_pool.tile([1, 1], f32)
                nc.vector.tensor_reduce(
                    out=neg_mx,
                    in_=scores,
                    axis=mybir.AxisListType.X,
                    op=mybir.AluOpType.max,
                    negate=True,
                )
                exps = sm_pool.tile([1, CHI], f32)
                sumexp = sm_pool.tile([1, 1], f32)
                nc.scalar.activation(
                    out=exps,
                    in_=scores,
                    func=mybir.ActivationFunctionType.Exp,
                    bias=neg_mx[:, 0:1],
                    scale=1.0,
                    accum_out=sumexp,
                )
                rsum = sm_pool.tile([1, 1], f32)
                nc.vector.reciprocal(rsum, sumexp)
                alpha = sm_pool.tile([1, CHI], f32)
                nc.vector.tensor_scalar_mul(alpha, exps, rsum)

                # broadcast alpha to all partitions: [128, CHI]
                b_psum = p_pool.tile([P, CHI], f32)
                nc.tensor.matmul(b_psum, ones_row, alpha, start=True, stop=True)
                alpha_bc = sm_pool.tile([P, CHI], f32)
                nc.scalar.copy(out=alpha_bc, in_=b_psum)

                # ---- stage 2: weighted sum over interleaved view ----
                out_t = o_pool.tile([P, F], f32)
                nc.vector.tensor_scalar_mul(out_t, t2[:, :, 0], alpha_bc[:, 0:1])
                for c in range(1, CHI):
                    nc.vector.scalar_tensor_tensor(
                        out=out_t,
                        in0=t2[:, :, c],
                        scalar=alpha_bc[:, c : c + 1],
                        in1=out_t,
                        op0=mybir.AluOpType.mult,
                        op1=mybir.AluOpType.add,
                    )

                nc.sync.dma_start(
                    out=y_d[b].rearrange("(p f) -> p f", p=P), in_=out_t
                )

    nc.compile()
    return nc


def _get_nc():
    if "nc" not in _CACHE:
        _CACHE["nc"] = _build_nc()
    return _CACHE["nc"]


def _get_runner():
    """Compile once and return f(x_global[64, CHI*D]) -> y_global[64, D].

    Mirrors concourse.bass2jax.run_bass_via_pjrt but caches the jitted
    executable so repeated kernel() calls don't re-trace/re-compile.
    """
    if "runner" in _CACHE:
        return _CACHE["runner"]

    import jax
    from jax.sharding import Mesh, PartitionSpec
    from jax.experimental.shard_map import shard_map
    from concourse import bass2jax, mybir

    nc = _get_nc()
    bass2jax.install_neuronx_cc_hook()

    partition_name = (
        nc.partition_id_tensor.name if nc.partition_id_tensor else None
    )
    in_names = []
    out_names = []
    out_avals = []
    zero_outs = []
    for alloc in nc.m.functions[0].allocations:
        if not isinstance(alloc, mybir.MemoryLocationSet):
            continue
        name = alloc.memorylocations[0].name
        if alloc.kind == "ExternalInput":
            if name != partition_name:
                in_names.append(name)
        elif alloc.kind == "ExternalOutput":
            shape = tuple(alloc.tensor_shape)
            dtype = mybir.dt.np(alloc.dtype)
            out_avals.append(jax.core.ShapedArray(shape, dtype))
            out_names.append(name)
            zero_outs.append(np.zeros(shape, dtype))
    n_params = len(in_names)
    n_outs = len(out_avals)
    in_names.extend(out_names)
    donate = tuple(range(n_params, n_params + n_outs))

    def _body(*args):
        operands = list(args)
        if partition_name is not None:
            operands.append(bass2jax.partition_id_tensor())
            in_full = tuple(in_names) + (partition_name,)
        else:
            in_full = tuple(in_names)
        outs = bass2jax._bass_exec_p.bind(
            *operands,
            out_avals=tuple(out_avals),
            in_names=in_full,
            out_names=tuple(out_names),
            lowering_input_output_aliases=(),
            sim_require_finite=True,
            sim_require_nnan=True,
            nc=nc,
        )
        return tuple(outs)

    devices = jax.devices()[:N_CORES]
    mesh = Mesh(np.asarray(devices), ("core",))
    in_specs = (PartitionSpec("core"),) * (n_params + n_outs)
    out_specs = (PartitionSpec("core"),) * len(out_names)
    sharded = jax.jit(
        shard_map(
            _body, mesh=mesh, in_specs=in_specs, out_specs=out_specs, check_rep=False
        ),
        donate_argnums=donate,
        keep_unused=True,
    )

    def run(xs):
        concat_zeros = [
            np.zeros((N_CORES * z.shape[0], *z.shape[1:]), z.dtype) for z in zero_outs
        ]
        return sharded(xs, *concat_zeros)[0]

    _CACHE["sharded"] = sharded
    _CACHE["mesh"] = mesh
    _CACHE["runner"] = run
    return run


def kernel(**inputs):
    x = np.ascontiguousarray(np.asarray(inputs["x"], dtype=np.float32))
    assert x.shape == (B, CHI, 64, 32, 32), x.shape
    xs = x.reshape(B, CHI * D)
    run = _get_runner()
    last_err = None
    for _attempt in range(3):
        try:
            out = np.asarray(run(xs))
            break
        except Exception as e:  # transient NRT device errors: retry
            last_err = e
    else:
        raise last_err
    return out.reshape(B, 64, 32, 32)
